# revision 1
# baseline (speedup 1.0000x reference)
"""TRN2 Bass kernel for nn_CosFreqEncoding: out = ((x @ W.T) @ cos_basis) / max.

Strategy: data-parallel over batch across 8 NeuronCores. Each core computes
its 512-row shard of both GEMMs in fp32r (e8m11, full TensorE rate), a local
max, one scalar AllReduce(max), then scales and writes its output shard.

Layouts (host-prepped so no on-chip transposes are needed):
  GEMM1: xfT[f, m] += W.T[l, f].T @ x.T[l, m]   (lhsT = W.T block, rhs = x.T)
  GEMM2: out[m, l2] += xfT[f, m].T @ cos[f, l2] (lhsT = xfT slice, rhs = cos)

Self-contained: hardcodes shapes from the problem spec.
"""
import numpy as np

import concourse.bass as bass
import concourse.bacc as bacc
import concourse.mybir as mybir
import concourse.tile as tile
import concourse.bass_utils as bass_utils

N_CORES = 8
B, L, F = 4096, 2048, 2074
FP = 2176               # F padded to 17 full 128-tiles
BS = B // N_CORES       # 512 batch rows per core
LT = L // 128           # 16 l-tiles (GEMM1 contraction)
FT = FP // 128          # 17 f-tiles
MT = BS // 128          # 4 m-tiles
CK = L // 512           # 4 output column chunks of 512
F32 = mybir.dt.float32
F32R = mybir.dt.float32r
NEG_INF = -3.0e38


def _to_fp32r(a: np.ndarray) -> np.ndarray:
    """Round fp32 to fp32r (e8m11): RNE at mantissa bit 12, low bits zeroed."""
    b = np.ascontiguousarray(a, dtype=np.float32).view(np.uint32).astype(np.uint64)
    b = b + 0x7FF + ((b >> 12) & 1)
    return (b & 0xFFFF_F000).astype(np.uint32).view(np.float32)


def _armax(nc, sp, dp, vm_slice, tag, q):
    """Local max of vm_slice -> scalar -> AllReduce(max); returns dram out."""
    g = sp.tile([1, 1], F32, name=f"g_{tag}")
    q.reduce_max(g[:], vm_slice, axis=mybir.AxisListType.XYZWC)
    cc_in = dp.tile([1], F32, name=f"ccin_{tag}")
    cc_out = dp.tile([1], F32, name=f"ccout_{tag}")
    q.dma_start(cc_in[:], g[:, 0])
    nc.gpsimd.collective_compute(
        "AllReduce", mybir.AluOpType.max,
        replica_groups=[list(range(N_CORES))],
        ins=[cc_in[:]], outs=[cc_out[:]])
    return cc_out


def _emit(nc, tc, xT, Wb, cosb, out, variant):
    with (
        tc.tile_pool(name="xp", bufs=1) as xp,
        tc.tile_pool(name="wp", bufs=4) as wp,
        tc.tile_pool(name="xfp", bufs=1) as xfp,
        tc.tile_pool(name="cp", bufs=12) as cp,
        tc.tile_pool(name="op", bufs=1) as op,
        tc.tile_pool(name="sp", bufs=1) as sp,
        tc.tile_pool(name="ps1", bufs=2, space="PSUM") as ps1,
        tc.tile_pool(name="ps2", bufs=6, space="PSUM") as ps2,
        tc.tile_pool(name="dp", bufs=1, space="DRAM") as dp,
    ):
        # DMA issuers round-robin: each engine owns its own DGE queue, and a
        # single queue's descriptor-gen (~600ns/transfer) caps at ~110GB/s.
        qs = [nc.sync, nc.scalar, nc.gpsimd]

        # resident x.T tiles: [128 l, 512 m] x 16
        xt = []
        for li in range(LT):
            t = xp.tile([128, BS], F32R, name=f"xt{li}")
            (nc.sync if li % 2 == 0 else nc.gpsimd).dma_start(t[:], xT[li])
            xt.append(t)

        if variant == "io":
            w0 = wp.tile([128, LT * 128], F32R, tag="w")
            nc.sync.dma_start(
                w0[:].rearrange("p (li b) -> p li b", li=LT), Wb[0])
            c0 = cp.tile([128, 512], F32R, tag="cos")
            nc.sync.dma_start(c0[:], cosb[0, 0])
            for li in range(4):
                nc.sync.dma_start(out[li * 128:(li + 1) * 128, 0:512],
                                  xt[li][:].bitcast(F32))
            nc.sync.dma_start(out[0:128, 512:640],
                              w0[:, 0:128].bitcast(F32))
            nc.sync.dma_start(out[0:128, 1024:1536], c0[:].bitcast(F32))
            return

        # GEMM1: xfT[f-tile] [128 f, 512 m]. W streamed as one 1MB DMA per
        # f-tile column (16 blocks) to amortize DGE descriptor-gen.
        xf = [xfp.tile([128, BS], F32R, name=f"xf{fi}") for fi in range(FT)]
        for fi in range(FT):
            ps = ps1.tile([128, BS], F32, tag="g1")
            wcol = wp.tile([128, LT * 128], F32R, tag="w")
            wv = wcol[:].rearrange("p (li b) -> p li b", li=LT)
            (nc.scalar if fi % 2 == 0 else nc.sync).dma_start(wv, Wb[fi])
            for li in range(LT):
                nc.tensor.matmul(ps[:], wcol[:, li * 128:(li + 1) * 128],
                                 xt[li][:],
                                 start=(li == 0), stop=(li == LT - 1))
            # cast+round fp32 -> fp32r while copying out of PSUM
            nc.vector.tensor_copy(xf[fi][:], ps[:])

        if variant == "g1":
            for ci in range(4):
                nc.sync.dma_start(out[0:128, ci * 512:(ci + 1) * 512],
                                  xf[ci][:].bitcast(F32))
            return

        # GEMM2 + fused local max
        ot = [op.tile([128, L], F32, name=f"ot{mi}") for mi in range(MT)]
        vmaxes = sp.tile([128, MT * CK], F32)
        for ci in range(CK):
            pst = [ps2.tile([128, 512], F32, tag="g2", name=f"ps2_{ci}_{mi}")
                   for mi in range(MT)]
            for fi in range(FT):
                c = cp.tile([128, 512], F32R, tag="cos")
                qs[(ci * FT + fi) % 2].dma_start(c[:], cosb[ci, fi])
                for mi in range(MT):
                    nc.tensor.matmul(
                        pst[mi][:], xf[fi][:, mi * 128:(mi + 1) * 128], c[:],
                        start=(fi == 0), stop=(fi == FT - 1))
            for mi in range(MT):
                idx = ci * MT + mi
                osl = ot[mi][:, ci * 512:(ci + 1) * 512]
                nc.vector.tensor_copy(osl, pst[mi][:])
                if variant != "g2a":
                    nc.vector.reduce_max(vmaxes[:, idx:idx + 1], osl,
                                         axis=mybir.AxisListType.X)
            if variant == "full" and ci == 1:
                # stage-1 AllReduce over chunks 0..1, hidden under chunks 2-3
                cc1_out = _armax(nc, sp, dp, vmaxes[:, 0:2 * MT], "s1",
                                 nc.gpsimd)

        if variant in ("nonorm", "g2a"):
            for mi in range(MT):
                nc.sync.dma_start(out[mi * 128:(mi + 1) * 128, :], ot[mi][:])
            return

        # stage-2 AllReduce over the last chunk's maxes, then combine
        cc2_out = _armax(nc, sp, dp, vmaxes[:, 2 * MT:CK * MT], "s2",
                         nc.gpsimd)
        gbc1 = sp.tile([128, 1], F32)
        nc.sync.dma_start(gbc1[:], cc1_out[:].partition_broadcast(128))
        gbc2 = sp.tile([128, 1], F32)
        nc.scalar.dma_start(gbc2[:], cc2_out[:].partition_broadcast(128))
        gbc = sp.tile([128, 1], F32)
        nc.vector.tensor_scalar_max(gbc[:], gbc1[:], gbc2[:, 0:1])
        rbc = sp.tile([128, 1], F32)
        nc.vector.reciprocal(rbc[:], gbc[:])

        # scale + store (chunked so DMA of one slice overlaps mul of the next)
        for mi in range(MT):
            for ci in range(CK):
                sl = slice(ci * 512, (ci + 1) * 512)
                nc.vector.tensor_scalar_mul(ot[mi][:, sl], ot[mi][:, sl],
                                            rbc[:, 0:1])
                qs[(mi * CK + ci) % 3].dma_start(
                    out[mi * 128:(mi + 1) * 128, sl], ot[mi][:, sl])


def _build(variant="full"):
    nc = bacc.Bacc("TRN2", target_bir_lowering=False, debug=False,
                   num_devices=N_CORES)
    xT = nc.dram_tensor("xT", [LT, 128, BS], F32R, kind="ExternalInput")
    Wb = nc.dram_tensor("Wb", [FT, 128, LT, 128], F32R, kind="ExternalInput")
    cosb = nc.dram_tensor("cosb", [CK, FT, 128, 512], F32R, kind="ExternalInput")
    out = nc.dram_tensor("out", [BS, L], F32, kind="ExternalOutput")
    with tile.TileContext(nc) as tc:
        _emit(nc, tc, xT, Wb, cosb, out, variant)
    nc.compile()
    return nc


_cached_nc = None


def _get_nc():
    global _cached_nc
    if _cached_nc is None:
        _cached_nc = _build()
    return _cached_nc


def _prep_inputs(x, W, cos_basis):
    x = np.ascontiguousarray(x, dtype=np.float32)
    W = np.ascontiguousarray(W, dtype=np.float32)
    cos = np.ascontiguousarray(cos_basis, dtype=np.float32)
    # pad freq dim to FP with zeros
    Wp = np.zeros((FP, L), dtype=np.float32)
    Wp[:F] = W
    cosp = np.zeros((FP, L), dtype=np.float32)
    cosp[:F] = cos
    # Wb[fi, p, li, b] = W.T[li*128+p, fi*128+b] = Wp[fi*128+b, li*128+p]
    Wb = np.ascontiguousarray(
        Wp.reshape(FT, 128, LT, 128).transpose(0, 3, 2, 1))
    Wb = _to_fp32r(Wb)
    # cosb[ci, fi, a, n] = cosp[fi*128+a, ci*512+n]
    cosb = np.ascontiguousarray(
        cosp.reshape(FT, 128, CK, 512).transpose(2, 0, 1, 3))
    cosb = _to_fp32r(cosb)
    xTs = []
    for i in range(N_CORES):
        sh = np.ascontiguousarray(x[i * BS:(i + 1) * BS].T)  # (L, BS)
        xTs.append(_to_fp32r(sh.reshape(LT, 128, BS)))
    return xTs, Wb, cosb


def kernel(x, W, cos_basis, _trace=False, _trace_kwargs=None):
    xTs, Wb, cosb = _prep_inputs(x, W, cos_basis)
    nc = _get_nc()
    in_maps = [{"xT": xTs[i], "Wb": Wb, "cosb": cosb} for i in range(N_CORES)]
    res = bass_utils.run_bass_kernel_spmd(
        nc, in_maps, core_ids=list(range(N_CORES)), trace=_trace,
        **(_trace_kwargs or {}))
    out = np.concatenate([res.results[i]["out"] for i in range(N_CORES)],
                         axis=0)
    if _trace:
        kernel.last_result = res
    return out

